# revision 8
# baseline (speedup 1.0000x reference)
"""Trainium2 Bass kernel for quantized Conv1D forward:
    y = x @ (w_q * scale) + bias
  x:     [4, 2048, 4096] f32
  w_q:   [4096, 16384] int32 (values in [-127, 126])
  scale: [16384] f32
  bias:  [16384] f32
  y:     [4, 2048, 16384] f32

Sharding: column-parallel over out_features across 8 cores (N=2048 each);
x replicated. Each core computes y_shard = x @ W_shard + bias_shard
independently (no collectives); host concatenates shards.

Device strategy (V2, single-pass fp16):
  - The dequantized weight W = w_q*scale is cast to fp16 ON HOST (rel err
    ~2.4e-4) and stays fully resident in SBUF (128 KB/partition).
  - x is cast to fp16 (rel err ~2.4e-4); one matmul pass accumulates the
    full K=4096 contraction in fp32 PSUM. Total rel err ~3.4e-4, far
    below the 2e-2 gate, at half the PE work of the old hi/lo x-split.
  - Per-channel scale is already folded into W, so the only post-op is a
    single fp32 bias add (DVE) from PSUM -> SBUF, then DMA out.
  - PE runs back-to-back fp16 matmuls (stationary = x^T tile via FWL-
    hidden LDWEIGHTS, moving = W rows, N=512 per PSUM bank).
"""

import numpy as np

import concourse.bass as bass
import concourse.mybir as mybir
import concourse.tile as tile
from concourse import bacc
from concourse.bass import ts
from concourse.bass_utils import run_bass_kernel_spmd

P = 128
N_CORES = 8


def build_nc(T, K, N, n_free=512, reps=1,
             x_bufs=3, o_bufs=3, p_bufs=None, x_dma_split=2, y_dma_split=1,
             mm_dt="fp16", n_sub=2, sub_order="B", swap_loop=False,
             timing_mode=False, y_dt="f32"):
    """Build the per-core Bass program.

    DRAM I/O (per core):
      xh:   [TB, P, KB, Tt] fp16  packed x^T tiles
      wh:   [P, KB, N]      fp16  scaled weight shard (w_q*scale), k on partitions
      bias: [N] f32
      y:    [T, N] f32 out

    timing_mode: y goes to an Internal DRAM scratch tensor (identical
    compute + DMA work) and a tiny [P,1] dummy output is emitted instead,
    so slope-timing calls don't churn 64 MB/core output buffers.
    """
    KB = K // P
    TB = T // P
    Tt = P
    NB = N // n_free
    mdt = {"fp16": mybir.dt.float16, "bf16": mybir.dt.bfloat16}[mm_dt]
    ydt = {"f32": mybir.dt.float32, "f16": mybir.dt.float16}[y_dt]

    nc = bacc.Bacc("TRN2", target_bir_lowering=False, debug=False)

    xh = nc.dram_tensor("xh", [TB, P, KB, Tt], mdt, kind="ExternalInput")
    wh = nc.dram_tensor("wh", [P, KB, N], mdt, kind="ExternalInput")
    bias_h = nc.dram_tensor("bias", [N], mybir.dt.float32, kind="ExternalInput")
    if timing_mode:
        y_h = nc.dram_tensor("y", [T, N], ydt, kind="Internal")
        ydum_h = nc.dram_tensor("ydum", [P, 1], ydt, kind="ExternalOutput")
    else:
        y_h = nc.dram_tensor("y", [T, N], ydt, kind="ExternalOutput")

    xh_ap = xh.ap()
    wh_ap = wh.ap()
    y_ap = y_h.ap().rearrange("(tb p) n -> tb p n", p=P)

    def bcast_ap(ap):
        # [N] dram vector -> [P, N] with step-0 partition dim for DMA broadcast
        return bass.AP(tensor=ap.tensor, offset=ap.offset, ap=[[0, P], *ap.ap])

    with tile.TileContext(nc) as tc:
        if p_bufs is None:
            p_bufs = 2 * NB
        with (
            tc.tile_pool(name="wpool", bufs=1) as wpool,
            tc.tile_pool(name="cpool", bufs=1) as cpool,
            tc.tile_pool(name="xpool", bufs=x_bufs) as xpool,
            tc.tile_pool(name="opool", bufs=o_bufs) as opool,
            tc.tile_pool(name="ppool", bufs=p_bufs, space="PSUM") as ppool,
        ):
            # resident scaled weights: [P, KB, N]
            w_res = wpool.tile([P, KB, N], mdt, name="w_res")
            for kb in range(KB):
                nc.sync.dma_start(out=w_res[:, kb], in_=wh_ap[:, kb])

            bias_b = cpool.tile([P, N], mybir.dt.float32, name="bias_b")
            nc.sync.dma_start(out=bias_b[:], in_=bcast_ap(bias_h.ap()))

            for tb in [t for _ in range(reps) for t in range(TB)]:
                xt = xpool.tile([P, KB, Tt], mdt, tag="xt")
                if x_dma_split == 1:
                    nc.sync.dma_start(out=xt[:], in_=xh_ap[tb])
                else:
                    assert KB % x_dma_split == 0
                    c = KB // x_dma_split
                    for d in range(x_dma_split):
                        nc.sync.dma_start(
                            out=xt[:, ts(d, c)],
                            in_=xh_ap[tb, :, ts(d, c)],
                        )

                psums = [
                    ppool.tile([P, n_free], mybir.dt.float32, tag="acc", name=f"ps{nb}")
                    for nb in range(NB)
                ]
                # n_sub>1: split each PSUM bank into n_sub independent
                # accumulation regions, raising stationary reuse from NB to
                # NB*n_sub MMs per weight load
                w_free = n_free // n_sub
                if n_sub > 1:
                    if sub_order == "A":  # slice-major: cycle banks, then halves
                        sl_iter = [(b, h) for h in range(n_sub) for b in range(NB)]
                    else:  # "B" bank-major: both halves of a bank back-to-back
                        sl_iter = [(b, h) for b in range(NB) for h in range(n_sub)]
                    mm_iter = [(kb, b, h) for kb in range(KB) for (b, h) in sl_iter]
                elif swap_loop:
                    mm_iter = [(kb, nb, 0) for nb in range(NB) for kb in range(KB)]
                else:
                    mm_iter = [(kb, nb, 0) for kb in range(KB) for nb in range(NB)]
                for kb, b, h in mm_iter:
                    nc.tensor.matmul(
                        psums[b][:, ts(h, w_free)],
                        lhsT=xt[:, kb, :],
                        rhs=w_res[:, kb, b * n_free + h * w_free:
                                  b * n_free + (h + 1) * w_free],
                        start=(kb == 0 and h == 0),
                        stop=(kb == KB - 1),
                        skip_group_check=(n_sub > 1),
                    )

                out_sb = opool.tile([P, N], ydt, tag="out")
                for nb in range(NB):
                    nc.vector.tensor_add(
                        out=out_sb[:, ts(nb, n_free)],
                        in0=psums[nb][:],
                        in1=bias_b[:, ts(nb, n_free)],
                    )
                if y_dma_split == 1:
                    nc.sync.dma_start(out=y_ap[tb], in_=out_sb[:])
                else:
                    c = N // y_dma_split
                    for d in range(y_dma_split):
                        nc.sync.dma_start(
                            out=y_ap[tb, :, ts(d, c)], in_=out_sb[:, ts(d, c)]
                        )
                if timing_mode and tb == TB - 1:
                    nc.sync.dma_start(out=ydum_h.ap(), in_=out_sb[:, 0:1])

    nc.compile()
    return nc


def pack_x(x2d, T, K, np_dt=np.float16):
    """[T, K] f32 -> [TB, P, KB, Tt] tiles of x^T in fp16."""
    TB, KB = T // P, K // P
    x_hi = x2d.astype(np_dt)
    # [T, K] -> [TB, Tt, KB, Pk] -> [TB, Pk, KB, Tt]
    return np.ascontiguousarray(
        x_hi.reshape(TB, P, KB, P).transpose(0, 3, 2, 1)
    )


def pack_w(w_shard_f32, K, N, np_dt=np.float16):
    """[K, N] f32 scaled weights -> [P, KB, N] fp16, k on partitions."""
    KB = K // P
    return np.ascontiguousarray(
        w_shard_f32.astype(np_dt).reshape(KB, P, N).transpose(1, 0, 2)
    )


_NC_CACHE = {}

# tuned on hardware (interleaved A/B slope ranking): x DMA in 4 chunks so
# MMs start as soon as the first kb-chunk lands; single 512-wide accumulation
# region per PSUM bank (n_sub=1) — LDWEIGHTS is FWL-hidden, so extra
# stationary reuse granularity only added dispatch overhead
# y is written to DRAM as fp16 (halves output DMA traffic; +~2.8e-4 rel
# rounding, total ~4e-4, 50x under the 2e-2 gate) and upcast to f32 on host
TUNED = dict(x_dma_split=4, n_sub=1, y_dt="f16")


def _get_nc(T, K, N):
    key = (T, K, N)
    if key not in _NC_CACHE:
        _NC_CACHE[key] = build_nc(T, K, N, **TUNED)
    return _NC_CACHE[key]


def kernel(x, w_q, scale, bias):
    x = np.asarray(x)
    w_q = np.asarray(w_q)
    scale = np.asarray(scale, dtype=np.float32)
    bias = np.asarray(bias, dtype=np.float32)
    B, Sq, K = x.shape
    K2, D_OUT = w_q.shape
    assert K2 == K
    T = B * Sq
    N = D_OUT // N_CORES

    nc = _get_nc(T, K, N)

    xh = pack_x(np.ascontiguousarray(x.reshape(T, K)), T, K)
    w_f32 = w_q.astype(np.float32) * scale[None, :]
    in_maps = []
    for c in range(N_CORES):
        sl = slice(c * N, (c + 1) * N)
        in_maps.append(
            {
                "xh": xh,
                "wh": pack_w(w_f32[:, sl], K, N),
                "bias": np.ascontiguousarray(bias[sl], dtype=np.float32),
            }
        )

    res = run_bass_kernel_spmd(nc, in_maps, core_ids=list(range(N_CORES)))
    y = np.concatenate([r["y"] for r in res.results], axis=1)
    return y.reshape(B, Sq, D_OUT).astype(np.float32)
